# revision 1
# baseline (speedup 1.0000x reference)
"""Trainium2 Bass kernel for nn_Loss_20933670601009 (gathered-prob NLL loss).

Strategy: the loss only touches 3 elements per (l, b) position (one gathered
prob from each of rule/token/reference tables), so instead of streaming the
full ~566MB of prob tensors through the cores, each core element-gathers the
1536 f32 values it needs straight from HBM with one indirect DMA, then does a
handful of tiny vector ops + Ln + reductions.

Sharding: data-parallel over L_a (128 rows -> 16 rows x 8 cores, 512
positions per core). Per-core partial sums are combined on the host
(sum of 8 scalars), which together with the on-device -1/32 scaling
reproduces mean-over-batch of per-sequence sums.
"""

import os
import sys

import numpy as np

for _p in ("/opt/trn_rl_repo", "/root/.axon_site/_ro/trn_rl_repo"):
    if os.path.isdir(_p) and _p not in sys.path:
        sys.path.insert(0, _p)

L_A, B = 128, 32
V_RULE, V_TOK, V_REF = 2048, 32000, 512
EPS = 1e-07
N_CORES = 8
L_SH = L_A // N_CORES            # 16 sequence rows per core
NPOS = L_SH * B                  # 512 positions per core
P = 128                          # SBUF partitions
J = NPOS // P                    # 4 positions per partition
SEG = (0, NPOS * V_RULE, NPOS * V_RULE + NPOS * V_TOK)
VS = (V_RULE, V_TOK, V_REF)
N_FLAT = NPOS * (V_RULE + V_TOK + V_REF)

_CACHE = {}


def _build():
    """Build + compile the per-core Bass module (same NEFF on all 8 cores)."""
    import concourse.bacc as bacc
    import concourse.bass as bass
    import concourse.mybir as mybir
    import concourse.tile as tile

    f32 = mybir.dt.float32
    i32 = mybir.dt.int32
    alu = mybir.AluOpType

    nc = bacc.Bacc(
        "TRN2",
        target_bir_lowering=False,
        debug=False,
        enable_asserts=False,
        num_devices=N_CORES,
    )

    # meta layout (int32 [128, 40]):
    #   cols  0:12  gt indices, component-major blocks of 4 (rule|token|ref)
    #   cols 12:16  mask as f32 bit pattern
    #   cols 16:28  segment-local base offsets  q*V[c]   (< 2^24: DVE int
    #               arithmetic is f32-backed, so offsets must stay exact in
    #               f32; the segment base goes in via element_offset instead)
    #   cols 28:40  segment-local top offsets   base + V[c]-1
    meta_d = nc.dram_tensor("meta", [P, 40], i32, kind="ExternalInput").ap()
    flat_d = nc.dram_tensor("probs_flat", [N_FLAT, 1], f32, kind="ExternalInput").ap()
    out_d = nc.dram_tensor("out", [1, 1], f32, kind="ExternalOutput").ap()

    with tile.TileContext(nc) as tc:
        with (
            tc.tile_pool(name="sb", bufs=1) as pool,
            tc.tile_pool(name="ps", bufs=1, space="PSUM") as psum,
        ):
            meta = pool.tile([P, 40], i32)
            nc.sync.dma_start(out=meta[:], in_=meta_d[:])
            gt = meta[:, 0:12]
            maskf = meta[:, 12:16].bitcast(f32)
            base = meta[:, 16:28]
            top = meta[:, 28:40]

            # offs = max(gt, 0) + base  (gt < V per problem spec, so no top
            # clamp needed; in-segment offsets stay < 2^24 so the f32-backed
            # DVE int math is exact)
            offs = pool.tile([P, 12], i32)
            nc.vector.tensor_scalar(
                out=offs[:], in0=gt, scalar1=0, scalar2=None, op0=alu.max
            )
            nc.vector.tensor_add(out=offs[:], in0=offs[:], in1=base)

            # validity: gt >= 0 as f32 {0.0, 1.0}
            vm = pool.tile([P, 12], f32)
            nc.vector.tensor_scalar(
                out=vm[:], in0=gt, scalar1=0, scalar2=None, op0=alu.is_ge
            )

            # element-gathers: HW consumes ONE offset per partition row and
            # reads out.free_size contiguous elements, so each gather must be
            # offsets [P,1] -> out [P,1]; 12 gathers cover 3 components x 4
            # positions/partition. Segment base rides in element_offset (an
            # exact int64 constant). Each gather lands in its own tile so the
            # scheduler never serializes them on tile reuse, and masked
            # per-column values are ready as the train progresses. Token goes
            # last: its partial sum is the only op left after the final
            # gather.
            gv = pool.tile([P, 12], f32)
            order = (0, 2, 1)  # rule, ref, token
            for c in order:
                for j in range(J):
                    col = 4 * c + j
                    gcj = pool.tile([P, 1], f32, tag=f"g{col}")
                    nc.gpsimd.indirect_dma_start(
                        out=gcj[:],
                        out_offset=None,
                        in_=flat_d[:],
                        in_offset=bass.IndirectOffsetOnAxis(
                            ap=offs[:, col:col + 1], axis=0
                        ),
                        element_offset=SEG[c],
                    )
                    nc.vector.tensor_mul(
                        out=gv[:, col:col + 1], in0=gcj[:], in1=vm[:, col:col + 1]
                    )
            s = pool.tile([P, J], f32)
            nc.vector.tensor_add(out=s[:], in0=gv[:, 0:4], in1=gv[:, 8:12])
            nc.vector.tensor_add(out=s[:], in0=s[:], in1=gv[:, 4:8])

            # prob += (prob < eps) * eps  (an add, not a clamp)
            t1 = pool.tile([P, J], f32)
            nc.vector.tensor_scalar(
                out=t1[:], in0=s[:], scalar1=EPS, scalar2=EPS,
                op0=alu.is_lt, op1=alu.mult,
            )
            nc.vector.tensor_add(out=s[:], in0=s[:], in1=t1[:])

            ln = pool.tile([P, J], f32)
            nc.scalar.activation(out=ln[:], in_=s[:], func=mybir.ActivationFunctionType.Ln)

            # masked row sums (tensor_tensor_reduce wedges the exec unit on
            # HW, so mul + reduce as separate ops)
            lm = pool.tile([P, J], f32)
            nc.vector.tensor_mul(out=lm[:], in0=ln[:], in1=maskf)
            rs = pool.tile([P, 1], f32)
            nc.vector.reduce_sum(out=rs[:], in_=lm[:], axis=mybir.AxisListType.X)

            # partition reduction via PE; weight -1/B folds negation + mean
            negw = pool.tile([P, 1], f32)
            nc.gpsimd.memset(negw[:], -1.0 / B)
            acc = psum.tile([1, 1], f32)
            nc.tensor.matmul(out=acc[:], lhsT=rs[:], rhs=negw[:], start=True, stop=True)
            res = pool.tile([1, 1], f32)
            nc.scalar.copy(out=res[:], in_=acc[:])
            nc.sync.dma_start(out=out_d[:], in_=res[:])

    nc.compile()
    return nc


def get_nc():
    if "nc" not in _CACHE:
        _CACHE["nc"] = _build()
    return _CACHE["nc"]


def make_in_maps(rule_probs, token_probs, reference_probs, ground_truth_actions, mask):
    """Shard the full inputs into 8 per-core input maps."""
    rule_probs = np.ascontiguousarray(np.asarray(rule_probs, dtype=np.float32))
    token_probs = np.ascontiguousarray(np.asarray(token_probs, dtype=np.float32))
    reference_probs = np.ascontiguousarray(np.asarray(reference_probs, dtype=np.float32))
    gt = np.asarray(ground_truth_actions, dtype=np.int32)
    mask = np.asarray(mask, dtype=np.int32)

    q = np.arange(NPOS, dtype=np.int64)
    base_cols = np.empty((P, 12), np.int32)
    top_cols = np.empty((P, 12), np.int32)
    for c in range(3):
        b = q * VS[c]  # segment-local: stays < 2^24 so DVE f32 math is exact
        base_cols[:, c * 4:(c + 1) * 4] = b.reshape(P, J).astype(np.int32)
        top_cols[:, c * 4:(c + 1) * 4] = (b + VS[c] - 1).reshape(P, J).astype(np.int32)

    in_maps = []
    for i in range(N_CORES):
        lo, hi = i * L_SH, (i + 1) * L_SH
        gt_sh = gt[lo:hi].reshape(NPOS, 3)
        meta = np.empty((P, 40), np.int32)
        for c in range(3):
            meta[:, c * 4:(c + 1) * 4] = gt_sh[:, c].reshape(P, J)
        meta[:, 12:16] = (
            mask[lo:hi].reshape(NPOS).astype(np.float32).view(np.int32).reshape(P, J)
        )
        meta[:, 16:28] = base_cols
        meta[:, 28:40] = top_cols
        probs_flat = np.concatenate(
            [
                rule_probs[lo:hi].reshape(-1),
                token_probs[lo:hi].reshape(-1),
                reference_probs[lo:hi].reshape(-1),
            ]
        )
        in_maps.append({"meta": meta, "probs_flat": probs_flat.reshape(-1, 1)})
    return in_maps


def run(inputs, trace=False, trace_cores=None):
    """Run on the 8 NeuronCores; returns (scalar ndarray, BassKernelResults)."""
    from concourse.bass_utils import run_bass_kernel_spmd

    nc = get_nc()
    in_maps = make_in_maps(**inputs)
    res = run_bass_kernel_spmd(
        nc,
        in_maps,
        core_ids=list(range(N_CORES)),
        trace=trace,
        trace_cores=trace_cores,
    )
    total = np.float64(0.0)
    for r in res.results:
        total += np.float64(r["out"].reshape(())[()])
    return np.asarray(total, dtype=np.float32), res


def kernel(**inputs) -> np.ndarray:
    out, _ = run(inputs)
    return out

